# revision 19
# baseline (speedup 1.0000x reference)
"""Multi-head cross-attention (self-attention variant) on 8 Trainium2 NeuronCores.

Problem: x[1,4096,1024]; Wq/Wk/Wv[1024,1024] -> 16 heads x 64 dim; softmax(QK^T/8)V;
merge heads; @ Wo + bo -> [1,4096,1024].

Design (v3, software-pipelined, no collective):
- Tensor-parallel over heads: core k owns heads (2k, 2k+1) = inner cols/rows
  [128k : 128k+128] of Wq/Wk/Wv/Wo. All matmul inputs in bf16 (1 cycle/row on
  the PE at any output width; final rel-err ~5e-3, under the 2e-2 gate).
- attn@V runs "flipped": out O[i-block 128, 65] = P_block^T @ [v_h | ones],
  costing 65 PE rows per (j-block, i-block) instead of 512; the ones column
  accumulates the softmax denominator (scores ~ N(0,1), exp safe without max
  subtraction). The 4 i-block accumulators share one PSUM bank (acc4: first
  matmul's start=True clears the whole bank, later regions start=False).
- j-swept flash accumulation: sweep k covers key-chunk k (4 j-blocks) for all
  16 (query-chunk, head) pairs; per pair-sweep one DVE add spills acc4 into a
  per-pair SBUF partial. The Act engine (256 x 1024-wide exp = 267us over all
  N^2 scores) is the bound; emission is software-pipelined per block:
  scores/exp of pair p + attnV/spill of pair p-1, with K/V projections of
  sweep k+1 trickled in sub-block pieces so the Act queue never starves.
- No inter-core collective: each core computes the partial output projection
  y_k = O_k @ Wo[128k:128k+128, :] for all 4096 rows (O^T via PE transposes),
  DMAs it out in bf16, and the HOST sums the 8 partials + bo.
"""
import numpy as np
from contextlib import ExitStack

N_CORES = 8
N = 4096          # sequence length
QD = 1024         # model dim
DH = 64           # head dim
HPC = 2           # heads per core
CPC = HPC * DH    # inner dims per core = 128
IC = 512          # chunk size (queries per chunk / keys per j-sweep)
NI = N // IC      # 8 chunks
NP = NI * HPC     # 16 (chunk, head) pairs
SCALE = DH ** -0.5
VW = DH + 1       # v block width per head incl. ones column (65)

_CACHE = {}


def _build(debug=False, repeat=1, single=False):
    from concourse import bacc, tile, mybir

    f32 = mybir.dt.float32
    bf16 = mybir.dt.bfloat16
    Exp = mybir.ActivationFunctionType.Exp

    nc = bacc.Bacc("TRN2", target_bir_lowering=False, debug=False,
                   enable_asserts=False, num_devices=1 if single else N_CORES)

    xt_d = nc.dram_tensor("xt", [QD, N], bf16, kind="ExternalInput").ap()
    wq_d = nc.dram_tensor("wq", [QD, CPC], bf16, kind="ExternalInput").ap()
    wk_d = nc.dram_tensor("wk", [QD, CPC], bf16, kind="ExternalInput").ap()
    wv_d = nc.dram_tensor("wv", [QD, CPC], bf16, kind="ExternalInput").ap()
    wo_d = nc.dram_tensor("wo", [CPC, QD], bf16, kind="ExternalInput").ap()
    id_d = nc.dram_tensor("ident", [128, 128], bf16, kind="ExternalInput").ap()
    y_d = nc.dram_tensor("y_out", [N, QD], bf16, kind="ExternalOutput").ap()

    with tile.TileContext(nc) as tc:
        with ExitStack() as ctx:
            sb = ctx.enter_context(tc.tile_pool(name="sb", bufs=1))
            pt_pool = ctx.enter_context(tc.tile_pool(name="pt", bufs=4))
            o_pool = ctx.enter_context(tc.tile_pool(name="osb", bufs=8))
            ot_pool = ctx.enter_context(tc.tile_pool(name="otsb", bufs=2))
            y_pool = ctx.enter_context(tc.tile_pool(name="ysb", bufs=2))
            r_pool = ctx.enter_context(tc.tile_pool(name="rcp", bufs=8))
            psS = ctx.enter_context(tc.tile_pool(name="psS", bufs=2, space="PSUM"))
            psA = ctx.enter_context(tc.tile_pool(name="psA", bufs=4, space="PSUM"))

            # --- static SBUF residents ---
            # x^T resident as one tile; QD-block t lives at cols [N*t, N*(t+1))
            xts_all = sb.tile([128, 8 * N], bf16, name="xts_all")
            xts = [xts_all[:, N * t:N * (t + 1)] for t in range(8)]
            qks = [sb.tile([128, 2 * IC], bf16, name=f"qk{c}") for c in range(NI)]
            vs = [sb.tile([128, 8 * VW], bf16, name=f"v{c}") for c in range(NI)]
            parts = [sb.tile([128, 4 * VW], f32, name=f"part{p}")
                     for p in range(NP)]
            wq_sb = sb.tile([128, QD], bf16)   # QD-block t at cols 128t
            wk_sb = sb.tile([128, QD], bf16)
            wv_sb = sb.tile([128, QD], bf16)
            wo_sb = sb.tile([128, QD], bf16)   # this core's 128 rows of Wo
            id_sb = sb.tile([128, 128], bf16)

            # --- prologue DMAs: one batched 3D-AP DMA per weight and per xt
            # chunk (DMA issue costs 565ns each on the SP sequencer, so count
            # matters). First K0/Q0 matmuls gate on wk/wq + xt chunk 0. ---
            # weights go through the Pool-issued (SWDGE) DMA ring so their
            # transfers run in parallel with the xt chunk loads on the SP ring
            def load_w(sb_t, d_t):
                nc.gpsimd.dma_start(
                    out=sb_t.rearrange("p (t w) -> p t w", w=CPC),
                    in_=d_t.rearrange("(t p) w -> p t w", p=128))

            def load_xt(c):
                nc.sync.dma_start(
                    out=xts_all.rearrange("p (t w) -> p t w",
                                          w=N)[:, :, IC * c:IC * (c + 1)],
                    in_=xt_d.rearrange("(t p) w -> p t w",
                                       p=128)[:, :, IC * c:IC * (c + 1)])
            load_w(wk_sb, wk_d)
            load_xt(0)
            load_w(wq_sb, wq_d)
            load_w(wv_sb, wv_d)
            for c in range(1, NI):
                load_xt(c)
            nc.gpsimd.dma_start(out=wo_sb[:, :], in_=wo_d[:, :])
            nc.gpsimd.dma_start(out=id_sb[:, :], in_=id_d[:, :])

            # ones columns of v tiles (col 64 of each 65-wide block)
            for c in range(NI):
                v3 = vs[c].rearrange("p (b w) -> p b w", w=VW)
                nc.vector.memset(v3[:, :, DH:DH + 1], 1.0)
            # zero the per-pair output partials
            for p in range(NP):
                nc.vector.memset(parts[p][:, :], 0.0)

            def proj_q(c):
                q_ps = psA.tile([128, IC], f32, tag="a", name="q_ps")
                for t in range(8):
                    nc.tensor.matmul(q_ps[:, :], wq_sb[:, 128 * t:128 * t + CPC],
                                     xts[t][:, IC * c:IC * (c + 1)],
                                     start=(t == 0), stop=(t == 7))
                nc.vector.tensor_copy(qks[c][:, 0:IC], q_ps[:, :])

            def proj_v_piece(c, b):
                # one of the four [128, 128] V blocks of chunk c
                v_ps = psA.tile([128, CPC], f32, tag="a", name="v_ps")
                for t in range(8):
                    nc.tensor.matmul(
                        v_ps[:, :],
                        xts[t][:, IC * c + 128 * b:IC * c + 128 * (b + 1)],
                        wv_sb[:, 128 * t:128 * t + CPC],
                        start=(t == 0), stop=(t == 7))
                for h in range(HPC):
                    nc.vector.tensor_copy(
                        vs[c][:, VW * (2 * b + h):VW * (2 * b + h) + DH],
                        v_ps[:, DH * h:DH * (h + 1)])

            # k-projection trickled in two halves (k_ps lives across 2 blocks)
            kproj_state = {}

            def proj_k_first(c):
                k_ps = psA.tile([128, IC], f32, tag="a", name="k_ps")
                for t in range(4):
                    nc.tensor.matmul(k_ps[:, :], wk_sb[:, 128 * t:128 * t + CPC],
                                     xts[t][:, IC * c:IC * (c + 1)],
                                     start=(t == 0), stop=False)
                kproj_state[c] = k_ps

            def proj_k_second(c):
                k_ps = kproj_state.pop(c)
                for t in range(4, 8):
                    nc.tensor.matmul(k_ps[:, :], wk_sb[:, 128 * t:128 * t + CPC],
                                     xts[t][:, IC * c:IC * (c + 1)],
                                     start=False, stop=(t == 7))
                nc.vector.tensor_copy(qks[c][:, IC:2 * IC], k_ps[:, :])

            def emit_scores(k, c, h, g2):
                s_ps = psS.tile([128, 2 * IC], f32, tag="s", name="s_ps")
                qt = qks[c][DH * h:DH * (h + 1), 0:IC]
                for u in range(2):
                    jj = 2 * g2 + u
                    nc.tensor.matmul(
                        s_ps[:, IC * u:IC * (u + 1)],
                        qks[k][DH * h:DH * (h + 1),
                               IC + 128 * jj:IC + 128 * (jj + 1)],
                        qt, start=True, stop=True)
                pt = pt_pool.tile([128, 2 * IC], bf16, name="pt")
                nc.scalar.activation(pt[:, :], s_ps[:, :], Exp, scale=SCALE)
                return pt

            def emit_attnv(state, g2):
                k, h = state["k"], state["h"]
                if g2 == 0:
                    state["acc4"] = psA.tile([128, IC], f32, tag="a",
                                             name="acc4")
                acc4 = state["acc4"]
                pt = state["pt"][g2]
                for u in range(2):
                    jj = 2 * g2 + u
                    for ib in range(4):
                        first = (g2 == 0 and u == 0 and ib == 0)
                        nc.tensor.matmul(
                            acc4[:, VW * ib:VW * (ib + 1)],
                            pt[:, IC * u + 128 * ib:IC * u + 128 * (ib + 1)],
                            vs[k][:, VW * (2 * jj + h):VW * (2 * jj + h + 1)],
                            start=first, stop=(g2 == 1 and u == 1),
                            skip_group_check=not first)

            def emit_finish(state, ot_tiles):
                k, c, h, pid = state["k"], state["c"], state["h"], state["pid"]
                acc4 = state["acc4"]
                nc.vector.tensor_add(parts[pid][:, 0:4 * VW],
                                     parts[pid][:, 0:4 * VW],
                                     acc4[:, 0:4 * VW])
                if k != NI - 1:
                    return
                # last sweep: normalize, transpose into O^T, then (h==1) the
                # partial output projection for this chunk
                if h == 0:
                    ot_tiles[c] = ot_pool.tile([128, IC], bf16, name="ot")
                ot_cur = ot_tiles[c]
                rcp4 = r_pool.tile([128, 4], f32, name="rcp4")
                nc.vector.reciprocal(
                    rcp4[:, :],
                    parts[pid].rearrange("p (b w) -> p b w", w=VW)[:, :,
                                                                  DH:DH + 1])
                for ib in range(4):
                    o_sb = o_pool.tile([128, DH], bf16, name="o_sb")
                    nc.vector.tensor_scalar_mul(
                        o_sb[:, :], parts[pid][:, VW * ib:VW * ib + DH],
                        rcp4[:, ib:ib + 1])
                    tr = psA.tile([DH, 128], bf16, tag="a", name="tr")
                    nc.tensor.transpose(tr[:, :], o_sb[:, :], id_sb[:, :])
                    nc.vector.tensor_copy(
                        ot_cur[DH * h:DH * (h + 1), 128 * ib:128 * (ib + 1)],
                        tr[:, :])
                if h == 1:
                    # one combined y tile + a single 3D-AP DMA per chunk
                    y_sb = y_pool.tile([128, 4 * QD], bf16, name="y_sb")
                    for ib in range(4):
                        for e in range(2):
                            y_ps = psA.tile([128, IC], f32, tag="a",
                                            name="y_ps")
                            nc.tensor.matmul(
                                y_ps[:, :], ot_cur[:, 128 * ib:128 * (ib + 1)],
                                wo_sb[:, IC * e:IC * (e + 1)],
                                start=True, stop=True)
                            dst = y_sb[:, QD * ib + IC * e:
                                       QD * ib + IC * (e + 1)]
                            # split PSUM->SBUF copies between DVE and Act:
                            # during the last sweep the DVE is the bottleneck
                            # while Act has slack; the final chunk drains
                            # after the last exp, so it all goes to Act
                            if e == 1 or c == NI - 1:
                                nc.scalar.copy(dst, y_ps[:, :])
                            else:
                                nc.vector.tensor_copy(dst, y_ps[:, :])
                    nc.sync.dma_start(
                        out=y_d[IC * c:IC * (c + 1), :].rearrange(
                            "(b p) w -> p b w", p=128),
                        in_=y_sb.rearrange("p (b w) -> p b w", w=QD))

            for _rep in range(repeat):
                ot_tiles = {}
                # prologue projections at full speed: K0, Q0 (V0 goes into
                # block 0's proj slot)
                proj_k_first(0)
                proj_k_second(0)
                proj_q(0)

                prev = None
                for p in range(NP * NI + 1):   # 128 pair blocks + 1 flush
                    cur = None
                    if p < NP * NI:
                        k, idx = divmod(p, NP)
                        c, h = divmod(idx, 2)
                        cur = {"k": k, "c": c, "h": h, "pid": idx, "pt": [None,
                                                                          None]}
                        # sweep-0 only: Q projection for chunk c, just in time
                        if k == 0 and h == 0 and c >= 1:
                            proj_q(c)
                        cur["pt"][0] = emit_scores(k, c, h, 0)
                    if prev is not None:
                        emit_attnv(prev, 0)
                    if p < NP * NI:
                        # trickled projections for sweep k+1 (at sweep 0 the
                        # even blocks carry Q projections, so trickle into the
                        # odd blocks there)
                        if k == 0 and idx == 0:
                            for b in range(4):
                                proj_v_piece(0, b)
                        if k < NI - 1:
                            kpos = (5, 7) if k == 0 else (4, 5)
                            vpos = (9, 11, 13, 15) if k == 0 else (8, 9, 10, 11)
                            if idx == kpos[0]:
                                proj_k_first(k + 1)
                            elif idx == kpos[1]:
                                proj_k_second(k + 1)
                            elif idx in vpos:
                                proj_v_piece(k + 1, vpos.index(idx))
                        cur["pt"][1] = emit_scores(k, c, h, 1)
                    if prev is not None:
                        emit_attnv(prev, 1)
                        emit_finish(prev, ot_tiles)
                    prev = cur
    nc.compile()
    return nc


def _get_nc():
    if "nc" not in _CACHE:
        _CACHE["nc"] = _build()
    return _CACHE["nc"]


def _in_maps(x, Wq, Wk, Wv, Wo):
    import ml_dtypes
    bf = ml_dtypes.bfloat16
    xt = np.ascontiguousarray(x.reshape(N, QD).T).astype(bf)
    ident = np.eye(128, dtype=np.float32).astype(bf)
    in_maps = []
    for k in range(N_CORES):
        cs = CPC * k
        in_maps.append({
            "xt": xt,
            "wq": np.ascontiguousarray(Wq[:, cs:cs + CPC]).astype(bf),
            "wk": np.ascontiguousarray(Wk[:, cs:cs + CPC]).astype(bf),
            "wv": np.ascontiguousarray(Wv[:, cs:cs + CPC]).astype(bf),
            "wo": np.ascontiguousarray(Wo[cs:cs + CPC, :]).astype(bf),
            "ident": ident,
        })
    return in_maps


def kernel(x, Wq, Wk, Wv, Wo, bo):
    from concourse.bass_utils import run_bass_kernel_spmd

    x = np.asarray(x, dtype=np.float32)
    Wq = np.asarray(Wq, dtype=np.float32)
    Wk = np.asarray(Wk, dtype=np.float32)
    Wv = np.asarray(Wv, dtype=np.float32)
    Wo = np.asarray(Wo, dtype=np.float32)
    bo = np.asarray(bo, dtype=np.float32)

    nc = _get_nc()
    res = run_bass_kernel_spmd(nc, _in_maps(x, Wq, Wk, Wv, Wo),
                               list(range(N_CORES)))
    y = np.zeros((N, QD), dtype=np.float32)
    for k in range(N_CORES):
        y += res.results[k]["y_out"].astype(np.float32)
    y = y + bo[None, :]
    return y.reshape(1, N, QD).astype(np.float32)


# revision 24
# speedup vs baseline: 1.0425x; 1.0425x over previous
"""Multi-head cross-attention (self-attention variant) on 8 Trainium2 NeuronCores.

Problem: x[1,4096,1024]; Wq/Wk/Wv[1024,1024] -> 16 heads x 64 dim; softmax(QK^T/8)V;
merge heads; @ Wo + bo -> [1,4096,1024].

Design (v3, software-pipelined, no collective):
- Tensor-parallel over heads: core k owns heads (2k, 2k+1) = inner cols/rows
  [128k : 128k+128] of Wq/Wk/Wv/Wo. All matmul inputs in bf16 (1 cycle/row on
  the PE at any output width; final rel-err ~5e-3, under the 2e-2 gate).
- attn@V runs "flipped": out O[i-block 128, 65] = P_block^T @ [v_h | ones],
  costing 65 PE rows per (j-block, i-block) instead of 512; the ones column
  accumulates the softmax denominator (scores ~ N(0,1), exp safe without max
  subtraction). The 4 i-block accumulators share one PSUM bank (acc4: first
  matmul's start=True clears the whole bank, later regions start=False).
- j-swept flash accumulation: sweep k covers key-chunk k (4 j-blocks) for all
  16 (query-chunk, head) pairs; per pair-sweep one DVE add spills acc4 into a
  per-pair SBUF partial. The Act engine (256 x 1024-wide exp = 267us over all
  N^2 scores) is the bound; emission is software-pipelined per block:
  scores/exp of pair p + attnV/spill of pair p-1, with K/V projections of
  sweep k+1 trickled in sub-block pieces so the Act queue never starves.
- No inter-core collective: each core computes the partial output projection
  y_k = O_k @ Wo[128k:128k+128, :] for all 4096 rows (O^T via PE transposes),
  DMAs it out in bf16, and the HOST sums the 8 partials + bo.
"""
import numpy as np
from contextlib import ExitStack

N_CORES = 8
N = 4096          # sequence length
QD = 1024         # model dim
DH = 64           # head dim
HPC = 2           # heads per core
CPC = HPC * DH    # inner dims per core = 128
IC = 512          # chunk size (queries per chunk / keys per j-sweep)
NI = N // IC      # 8 chunks
NP = NI * HPC     # 16 (chunk, head) pairs
SCALE = DH ** -0.5
VW = DH + 1       # v block width per head incl. ones column (65)

_CACHE = {}


def _build(debug=False, repeat=1, single=False):
    from concourse import bacc, tile, mybir

    f32 = mybir.dt.float32
    bf16 = mybir.dt.bfloat16
    Exp = mybir.ActivationFunctionType.Exp

    nc = bacc.Bacc("TRN2", target_bir_lowering=False, debug=False,
                   enable_asserts=False, num_devices=1 if single else N_CORES)

    xt_d = nc.dram_tensor("xt", [QD, N], bf16, kind="ExternalInput").ap()
    wq_d = nc.dram_tensor("wq", [QD, CPC], bf16, kind="ExternalInput").ap()
    wk_d = nc.dram_tensor("wk", [QD, CPC], bf16, kind="ExternalInput").ap()
    wv_d = nc.dram_tensor("wv", [QD, CPC], bf16, kind="ExternalInput").ap()
    wo_d = nc.dram_tensor("wo", [CPC, QD], bf16, kind="ExternalInput").ap()
    id_d = nc.dram_tensor("ident", [128, 128], bf16, kind="ExternalInput").ap()
    y_d = nc.dram_tensor("y_out", [N, QD], bf16, kind="ExternalOutput").ap()

    with tile.TileContext(nc) as tc:
        with ExitStack() as ctx:
            sb = ctx.enter_context(tc.tile_pool(name="sb", bufs=1))
            pt_pool = ctx.enter_context(tc.tile_pool(name="pt", bufs=4))
            o_pool = ctx.enter_context(tc.tile_pool(name="osb", bufs=8))
            ot_pool = ctx.enter_context(tc.tile_pool(name="otsb", bufs=2))
            y_pool = ctx.enter_context(tc.tile_pool(name="ysb", bufs=2))
            r_pool = ctx.enter_context(tc.tile_pool(name="rcp", bufs=8))
            psS = ctx.enter_context(tc.tile_pool(name="psS", bufs=2, space="PSUM"))
            psA = ctx.enter_context(tc.tile_pool(name="psA", bufs=4, space="PSUM"))

            # --- static SBUF residents ---
            # x^T resident as one tile; QD-block t lives at cols [N*t, N*(t+1))
            xts_all = sb.tile([128, 8 * N], bf16, name="xts_all")
            xts = [xts_all[:, N * t:N * (t + 1)] for t in range(8)]
            qks = [sb.tile([128, 2 * IC], bf16, name=f"qk{c}") for c in range(NI)]
            vs = [sb.tile([128, 8 * VW], bf16, name=f"v{c}") for c in range(NI)]
            parts = [sb.tile([128, 4 * VW], f32, name=f"part{p}")
                     for p in range(NP)]
            wq_sb = sb.tile([128, QD], bf16)   # QD-block t at cols 128t
            wk_sb = sb.tile([128, QD], bf16)
            wv_sb = sb.tile([128, QD], bf16)
            wo_sb = sb.tile([128, QD], bf16)   # this core's 128 rows of Wo
            id_sb = sb.tile([128, 128], bf16)

            # --- prologue DMAs: one batched 3D-AP DMA per weight and per xt
            # chunk (DMA issue costs 565ns each on the SP sequencer, so count
            # matters). First K0/Q0 matmuls gate on wk/wq + xt chunk 0. ---
            def load_w(sb_t, d_t):
                nc.sync.dma_start(
                    out=sb_t.rearrange("p (t w) -> p t w", w=CPC),
                    in_=d_t.rearrange("(t p) w -> p t w", p=128))

            def load_xt(c):
                nc.sync.dma_start(
                    out=xts_all.rearrange("p (t w) -> p t w",
                                          w=N)[:, :, IC * c:IC * (c + 1)],
                    in_=xt_d.rearrange("(t p) w -> p t w",
                                       p=128)[:, :, IC * c:IC * (c + 1)])
            load_xt(0)
            load_w(wk_sb, wk_d)
            load_w(wq_sb, wq_d)
            load_w(wv_sb, wv_d)
            for c in range(1, NI):
                load_xt(c)
            nc.sync.dma_start(out=wo_sb[:, :], in_=wo_d[:, :])
            nc.sync.dma_start(out=id_sb[:, :], in_=id_d[:, :])

            # ones columns of v tiles (col 64 of each 65-wide block)
            for c in range(NI):
                v3 = vs[c].rearrange("p (b w) -> p b w", w=VW)
                nc.vector.memset(v3[:, :, DH:DH + 1], 1.0)
            # zero the per-pair output partials
            for p in range(NP):
                nc.vector.memset(parts[p][:, :], 0.0)

            # q-projection trickled in two halves (q_ps lives across 2 blocks)
            qproj_state = {}

            def proj_q_first(c):
                q_ps = psA.tile([128, IC], f32, tag="a", name="q_ps")
                for t in range(4):
                    nc.tensor.matmul(q_ps[:, :], wq_sb[:, 128 * t:128 * t + CPC],
                                     xts[t][:, IC * c:IC * (c + 1)],
                                     start=(t == 0), stop=False)
                qproj_state[c] = q_ps

            def proj_q_second(c):
                q_ps = qproj_state.pop(c)
                for t in range(4, 8):
                    nc.tensor.matmul(q_ps[:, :], wq_sb[:, 128 * t:128 * t + CPC],
                                     xts[t][:, IC * c:IC * (c + 1)],
                                     start=False, stop=(t == 7))
                nc.vector.tensor_copy(qks[c][:, 0:IC], q_ps[:, :])

            def proj_q(c):
                proj_q_first(c)
                proj_q_second(c)

            def proj_k_half(c, half):
                # half a key chunk (2 j-blocks): only these gate the first
                # score groups of a sweep
                k_ps = psA.tile([128, IC // 2], f32, tag="a", name="k_ps2")
                lo = (IC // 2) * half
                for t in range(8):
                    nc.tensor.matmul(k_ps[:, :], wk_sb[:, 128 * t:128 * t + CPC],
                                     xts[t][:, IC * c + lo:IC * c + lo + IC // 2],
                                     start=(t == 0), stop=(t == 7))
                nc.vector.tensor_copy(qks[c][:, IC + lo:IC + lo + IC // 2],
                                      k_ps[:, :])

            def proj_v_piece(c, b):
                # one of the four [128, 128] V blocks of chunk c
                v_ps = psA.tile([128, CPC], f32, tag="a", name="v_ps")
                for t in range(8):
                    nc.tensor.matmul(
                        v_ps[:, :],
                        xts[t][:, IC * c + 128 * b:IC * c + 128 * (b + 1)],
                        wv_sb[:, 128 * t:128 * t + CPC],
                        start=(t == 0), stop=(t == 7))
                for h in range(HPC):
                    nc.vector.tensor_copy(
                        vs[c][:, VW * (2 * b + h):VW * (2 * b + h) + DH],
                        v_ps[:, DH * h:DH * (h + 1)])

            # k-projection trickled in two halves (k_ps lives across 2 blocks)
            kproj_state = {}

            def proj_k_first(c):
                k_ps = psA.tile([128, IC], f32, tag="a", name="k_ps")
                for t in range(4):
                    nc.tensor.matmul(k_ps[:, :], wk_sb[:, 128 * t:128 * t + CPC],
                                     xts[t][:, IC * c:IC * (c + 1)],
                                     start=(t == 0), stop=False)
                kproj_state[c] = k_ps

            def proj_k_second(c):
                k_ps = kproj_state.pop(c)
                for t in range(4, 8):
                    nc.tensor.matmul(k_ps[:, :], wk_sb[:, 128 * t:128 * t + CPC],
                                     xts[t][:, IC * c:IC * (c + 1)],
                                     start=False, stop=(t == 7))
                nc.vector.tensor_copy(qks[c][:, IC:2 * IC], k_ps[:, :])

            def emit_scores(k, c, h, g2):
                s_ps = psS.tile([128, 2 * IC], f32, tag="s", name="s_ps")
                qt = qks[c][DH * h:DH * (h + 1), 0:IC]
                for u in range(2):
                    jj = 2 * g2 + u
                    nc.tensor.matmul(
                        s_ps[:, IC * u:IC * (u + 1)],
                        qks[k][DH * h:DH * (h + 1),
                               IC + 128 * jj:IC + 128 * (jj + 1)],
                        qt, start=True, stop=True)
                pt = pt_pool.tile([128, 2 * IC], bf16, name="pt")
                nc.scalar.activation(pt[:, :], s_ps[:, :], Exp, scale=SCALE)
                return pt

            def emit_attnv(state, g2):
                k, h = state["k"], state["h"]
                if g2 == 0:
                    state["acc4"] = psA.tile([128, IC], f32, tag="a",
                                             name="acc4")
                acc4 = state["acc4"]
                pt = state["pt"][g2]
                for u in range(2):
                    jj = 2 * g2 + u
                    for ib in range(4):
                        first = (g2 == 0 and u == 0 and ib == 0)
                        nc.tensor.matmul(
                            acc4[:, VW * ib:VW * (ib + 1)],
                            pt[:, IC * u + 128 * ib:IC * u + 128 * (ib + 1)],
                            vs[k][:, VW * (2 * jj + h):VW * (2 * jj + h + 1)],
                            start=first, stop=(g2 == 1 and u == 1),
                            skip_group_check=not first)

            def emit_finish(state, ot_tiles):
                k, c, h, pid = state["k"], state["c"], state["h"], state["pid"]
                acc4 = state["acc4"]
                nc.vector.tensor_add(parts[pid][:, 0:4 * VW],
                                     parts[pid][:, 0:4 * VW],
                                     acc4[:, 0:4 * VW])
                if k != NI - 1:
                    return
                # last sweep: normalize, transpose into O^T, then (h==1) the
                # partial output projection for this chunk
                if h == 0:
                    ot_tiles[c] = ot_pool.tile([128, IC], bf16, name="ot")
                ot_cur = ot_tiles[c]
                rcp4 = r_pool.tile([128, 4], f32, name="rcp4")
                nc.vector.reciprocal(
                    rcp4[:, :],
                    parts[pid].rearrange("p (b w) -> p b w", w=VW)[:, :,
                                                                  DH:DH + 1])
                for ib in range(4):
                    o_sb = o_pool.tile([128, DH], bf16, name="o_sb")
                    nc.vector.tensor_scalar_mul(
                        o_sb[:, :], parts[pid][:, VW * ib:VW * ib + DH],
                        rcp4[:, ib:ib + 1])
                    tr = psA.tile([DH, 128], bf16, tag="a", name="tr")
                    nc.tensor.transpose(tr[:, :], o_sb[:, :], id_sb[:, :])
                    dst = ot_cur[DH * h:DH * (h + 1), 128 * ib:128 * (ib + 1)]
                    if h == 1:
                        nc.scalar.copy(dst, tr[:, :])
                    else:
                        nc.vector.tensor_copy(dst, tr[:, :])
                if h == 1:
                    # one combined y tile + a single 3D-AP DMA per chunk
                    y_sb = y_pool.tile([128, 4 * QD], bf16, name="y_sb")
                    for ib in range(4):
                        for e in range(2):
                            y_ps = psA.tile([128, IC], f32, tag="a",
                                            name="y_ps")
                            nc.tensor.matmul(
                                y_ps[:, :], ot_cur[:, 128 * ib:128 * (ib + 1)],
                                wo_sb[:, IC * e:IC * (e + 1)],
                                start=True, stop=True)
                            dst = y_sb[:, QD * ib + IC * e:
                                       QD * ib + IC * (e + 1)]
                            # split PSUM->SBUF copies between DVE and Act:
                            # during the last sweep the DVE is the bottleneck
                            # while Act has slack; the final chunk drains
                            # after the last exp, so it all goes to Act
                            if e == 1 or c == NI - 1:
                                nc.scalar.copy(dst, y_ps[:, :])
                            else:
                                nc.vector.tensor_copy(dst, y_ps[:, :])
                    nc.sync.dma_start(
                        out=y_d[IC * c:IC * (c + 1), :].rearrange(
                            "(b p) w -> p b w", p=128),
                        in_=y_sb.rearrange("p (b w) -> p b w", w=QD))

            for _rep in range(repeat):
                ot_tiles = {}
                # prologue: only what the first score group needs — the first
                # half of K0 (j-blocks 0,1) and all of Q0; K0's second half,
                # V0 and later Q's trickle into the block stream
                proj_k_half(0, 0)
                proj_q(0)

                prev = None
                for p in range(NP * NI + 1):   # 128 pair blocks + 1 flush
                    cur = None
                    if p < NP * NI:
                        k, idx = divmod(p, NP)
                        c, h = divmod(idx, 2)
                        cur = {"k": k, "c": c, "h": h, "pid": idx, "pt": [None,
                                                                          None]}
                        cur["pt"][0] = emit_scores(k, c, h, 0)
                    if prev is not None:
                        emit_attnv(prev, 0)
                    if p < NP * NI:
                        # trickled projections (sweep 0 also carries K0's
                        # second half, V0, and the Q projections for chunks
                        # 1..7, each split across two blocks just ahead of
                        # first use)
                        if k == 0:
                            if idx == 0:
                                proj_k_half(0, 1)
                                for b in range(4):
                                    proj_v_piece(0, b)
                            cq = idx // 2 + 1
                            if cq < NI:
                                if idx % 2 == 0:
                                    proj_q_first(cq)
                                else:
                                    proj_q_second(cq)
                        if k < NI - 1:
                            kpos = (5, 7) if k == 0 else (4, 5)
                            vpos = (9, 11, 13, 15) if k == 0 else (8, 9, 10, 11)
                            if idx == kpos[0]:
                                proj_k_first(k + 1)
                            elif idx == kpos[1]:
                                proj_k_second(k + 1)
                            elif idx in vpos:
                                proj_v_piece(k + 1, vpos.index(idx))
                        cur["pt"][1] = emit_scores(k, c, h, 1)
                    if prev is not None:
                        emit_attnv(prev, 1)
                        emit_finish(prev, ot_tiles)
                    prev = cur
    nc.compile()
    return nc


def _get_nc():
    if "nc" not in _CACHE:
        _CACHE["nc"] = _build()
    return _CACHE["nc"]


def _in_maps(x, Wq, Wk, Wv, Wo):
    import ml_dtypes
    bf = ml_dtypes.bfloat16
    xt = np.ascontiguousarray(x.reshape(N, QD).T).astype(bf)
    ident = np.eye(128, dtype=np.float32).astype(bf)
    in_maps = []
    for k in range(N_CORES):
        cs = CPC * k
        in_maps.append({
            "xt": xt,
            "wq": np.ascontiguousarray(Wq[:, cs:cs + CPC]).astype(bf),
            "wk": np.ascontiguousarray(Wk[:, cs:cs + CPC]).astype(bf),
            "wv": np.ascontiguousarray(Wv[:, cs:cs + CPC]).astype(bf),
            "wo": np.ascontiguousarray(Wo[cs:cs + CPC, :]).astype(bf),
            "ident": ident,
        })
    return in_maps


def kernel(x, Wq, Wk, Wv, Wo, bo):
    from concourse.bass_utils import run_bass_kernel_spmd

    x = np.asarray(x, dtype=np.float32)
    Wq = np.asarray(Wq, dtype=np.float32)
    Wk = np.asarray(Wk, dtype=np.float32)
    Wv = np.asarray(Wv, dtype=np.float32)
    Wo = np.asarray(Wo, dtype=np.float32)
    bo = np.asarray(bo, dtype=np.float32)

    nc = _get_nc()
    res = run_bass_kernel_spmd(nc, _in_maps(x, Wq, Wk, Wv, Wo),
                               list(range(N_CORES)))
    y = np.zeros((N, QD), dtype=np.float32)
    for k in range(N_CORES):
        y += res.results[k]["y_out"].astype(np.float32)
    y = y + bo[None, :]
    return y.reshape(1, N, QD).astype(np.float32)


# revision 25
# speedup vs baseline: 1.0774x; 1.0335x over previous
"""Multi-head cross-attention (self-attention variant) on 8 Trainium2 NeuronCores.

Problem: x[1,4096,1024]; Wq/Wk/Wv[1024,1024] -> 16 heads x 64 dim; softmax(QK^T/8)V;
merge heads; @ Wo + bo -> [1,4096,1024].

Design (v3, software-pipelined, no collective):
- Tensor-parallel over heads: core k owns heads (2k, 2k+1) = inner cols/rows
  [128k : 128k+128] of Wq/Wk/Wv/Wo. All matmul inputs in bf16 (1 cycle/row on
  the PE at any output width; final rel-err ~5e-3, under the 2e-2 gate).
- attn@V runs "flipped": out O[i-block 128, 65] = P_block^T @ [v_h | ones],
  costing 65 PE rows per (j-block, i-block) instead of 512; the ones column
  accumulates the softmax denominator (scores ~ N(0,1), exp safe without max
  subtraction). The 4 i-block accumulators share one PSUM bank (acc4: first
  matmul's start=True clears the whole bank, later regions start=False).
- j-swept flash accumulation: sweep k covers key-chunk k (4 j-blocks) for all
  16 (query-chunk, head) pairs; per pair-sweep one DVE add spills acc4 into a
  per-pair SBUF partial. The Act engine (256 x 1024-wide exp = 267us over all
  N^2 scores) is the bound; emission is software-pipelined per block:
  scores/exp of pair p + attnV/spill of pair p-1, with K/V projections of
  sweep k+1 trickled in sub-block pieces so the Act queue never starves.
- No inter-core collective: each core computes the partial output projection
  y_k = O_k @ Wo[128k:128k+128, :] for all 4096 rows (O^T via PE transposes),
  DMAs it out in bf16, and the HOST sums the 8 partials + bo.
"""
import numpy as np
from contextlib import ExitStack

N_CORES = 8
N = 4096          # sequence length
QD = 1024         # model dim
DH = 64           # head dim
HPC = 2           # heads per core
CPC = HPC * DH    # inner dims per core = 128
IC = 512          # chunk size (queries per chunk / keys per j-sweep)
NI = N // IC      # 8 chunks
NP = NI * HPC     # 16 (chunk, head) pairs
SCALE = DH ** -0.5
VW = DH + 1       # v block width per head incl. ones column (65)

_CACHE = {}


def _build(debug=False, repeat=1, single=False):
    from concourse import bacc, tile, mybir

    f32 = mybir.dt.float32
    bf16 = mybir.dt.bfloat16
    Exp = mybir.ActivationFunctionType.Exp

    nc = bacc.Bacc("TRN2", target_bir_lowering=False, debug=False,
                   enable_asserts=False, num_devices=1 if single else N_CORES)

    xt_d = nc.dram_tensor("xt", [QD, N], bf16, kind="ExternalInput").ap()
    wq_d = nc.dram_tensor("wq", [QD, CPC], bf16, kind="ExternalInput").ap()
    wk_d = nc.dram_tensor("wk", [QD, CPC], bf16, kind="ExternalInput").ap()
    wv_d = nc.dram_tensor("wv", [QD, CPC], bf16, kind="ExternalInput").ap()
    wo_d = nc.dram_tensor("wo", [CPC, QD], bf16, kind="ExternalInput").ap()
    id_d = nc.dram_tensor("ident", [128, 128], bf16, kind="ExternalInput").ap()
    y_d = nc.dram_tensor("y_out", [N, QD], bf16, kind="ExternalOutput").ap()

    with tile.TileContext(nc) as tc:
        with ExitStack() as ctx:
            sb = ctx.enter_context(tc.tile_pool(name="sb", bufs=1))
            pt_pool = ctx.enter_context(tc.tile_pool(name="pt", bufs=4))
            o_pool = ctx.enter_context(tc.tile_pool(name="osb", bufs=8))
            ot_pool = ctx.enter_context(tc.tile_pool(name="otsb", bufs=2))
            y_pool = ctx.enter_context(tc.tile_pool(name="ysb", bufs=2))
            r_pool = ctx.enter_context(tc.tile_pool(name="rcp", bufs=8))
            psS = ctx.enter_context(tc.tile_pool(name="psS", bufs=2, space="PSUM"))
            psA = ctx.enter_context(tc.tile_pool(name="psA", bufs=4, space="PSUM"))

            # --- static SBUF residents ---
            # x^T resident as one tile; QD-block t lives at cols [N*t, N*(t+1))
            xts_all = sb.tile([128, 8 * N], bf16, name="xts_all")
            xts = [xts_all[:, N * t:N * (t + 1)] for t in range(8)]
            qks = [sb.tile([128, 2 * IC], bf16, name=f"qk{c}") for c in range(NI)]
            vs = [sb.tile([128, 8 * VW], bf16, name=f"v{c}") for c in range(NI)]
            parts = [sb.tile([128, 4 * VW], f32, name=f"part{p}")
                     for p in range(NP)]
            wq_sb = sb.tile([128, QD], bf16)   # QD-block t at cols 128t
            wk_sb = sb.tile([128, QD], bf16)
            wv_sb = sb.tile([128, QD], bf16)
            wo_sb = sb.tile([128, QD], bf16)   # this core's 128 rows of Wo
            id_sb = sb.tile([128, 128], bf16)

            # --- prologue DMAs: one batched 3D-AP DMA per weight and per xt
            # chunk (DMA issue costs 565ns each on the SP sequencer, so count
            # matters). First K0/Q0 matmuls gate on wk/wq + xt chunk 0. ---
            def load_w(sb_t, d_t):
                nc.sync.dma_start(
                    out=sb_t.rearrange("p (t w) -> p t w", w=CPC),
                    in_=d_t.rearrange("(t p) w -> p t w", p=128))

            def load_xt(c):
                nc.sync.dma_start(
                    out=xts_all.rearrange("p (t w) -> p t w",
                                          w=N)[:, :, IC * c:IC * (c + 1)],
                    in_=xt_d.rearrange("(t p) w -> p t w",
                                       p=128)[:, :, IC * c:IC * (c + 1)])
            load_w(wk_sb, wk_d)
            load_xt(0)
            load_w(wq_sb, wq_d)
            load_w(wv_sb, wv_d)
            for c in range(1, NI):
                load_xt(c)
            nc.sync.dma_start(out=wo_sb[:, :], in_=wo_d[:, :])
            nc.sync.dma_start(out=id_sb[:, :], in_=id_d[:, :])

            # ones columns of v tiles (col 64 of each 65-wide block)
            for c in range(NI):
                v3 = vs[c].rearrange("p (b w) -> p b w", w=VW)
                nc.vector.memset(v3[:, :, DH:DH + 1], 1.0)
            # zero the per-pair output partials
            for p in range(NP):
                nc.vector.memset(parts[p][:, :], 0.0)

            # q-projection trickled in two halves (q_ps lives across 2 blocks)
            qproj_state = {}

            def proj_q_first(c):
                q_ps = psA.tile([128, IC], f32, tag="a", name="q_ps")
                for t in range(4):
                    nc.tensor.matmul(q_ps[:, :], wq_sb[:, 128 * t:128 * t + CPC],
                                     xts[t][:, IC * c:IC * (c + 1)],
                                     start=(t == 0), stop=False)
                qproj_state[c] = q_ps

            def proj_q_second(c):
                q_ps = qproj_state.pop(c)
                for t in range(4, 8):
                    nc.tensor.matmul(q_ps[:, :], wq_sb[:, 128 * t:128 * t + CPC],
                                     xts[t][:, IC * c:IC * (c + 1)],
                                     start=False, stop=(t == 7))
                nc.vector.tensor_copy(qks[c][:, 0:IC], q_ps[:, :])

            def proj_q(c):
                proj_q_first(c)
                proj_q_second(c)

            def proj_k_half(c, half):
                # half a key chunk (2 j-blocks): only these gate the first
                # score groups of a sweep
                k_ps = psA.tile([128, IC // 2], f32, tag="a", name="k_ps2")
                lo = (IC // 2) * half
                for t in range(8):
                    nc.tensor.matmul(k_ps[:, :], wk_sb[:, 128 * t:128 * t + CPC],
                                     xts[t][:, IC * c + lo:IC * c + lo + IC // 2],
                                     start=(t == 0), stop=(t == 7))
                nc.vector.tensor_copy(qks[c][:, IC + lo:IC + lo + IC // 2],
                                      k_ps[:, :])

            def proj_v_piece(c, b):
                # one of the four [128, 128] V blocks of chunk c
                v_ps = psA.tile([128, CPC], f32, tag="a", name="v_ps")
                for t in range(8):
                    nc.tensor.matmul(
                        v_ps[:, :],
                        xts[t][:, IC * c + 128 * b:IC * c + 128 * (b + 1)],
                        wv_sb[:, 128 * t:128 * t + CPC],
                        start=(t == 0), stop=(t == 7))
                for h in range(HPC):
                    nc.vector.tensor_copy(
                        vs[c][:, VW * (2 * b + h):VW * (2 * b + h) + DH],
                        v_ps[:, DH * h:DH * (h + 1)])

            # k-projection trickled in two halves (k_ps lives across 2 blocks)
            kproj_state = {}

            def proj_k_first(c):
                k_ps = psA.tile([128, IC], f32, tag="a", name="k_ps")
                for t in range(4):
                    nc.tensor.matmul(k_ps[:, :], wk_sb[:, 128 * t:128 * t + CPC],
                                     xts[t][:, IC * c:IC * (c + 1)],
                                     start=(t == 0), stop=False)
                kproj_state[c] = k_ps

            def proj_k_second(c):
                k_ps = kproj_state.pop(c)
                for t in range(4, 8):
                    nc.tensor.matmul(k_ps[:, :], wk_sb[:, 128 * t:128 * t + CPC],
                                     xts[t][:, IC * c:IC * (c + 1)],
                                     start=False, stop=(t == 7))
                nc.vector.tensor_copy(qks[c][:, IC:2 * IC], k_ps[:, :])

            def emit_scores(k, c, h, g2):
                s_ps = psS.tile([128, 2 * IC], f32, tag="s", name="s_ps")
                qt = qks[c][DH * h:DH * (h + 1), 0:IC]
                for u in range(2):
                    jj = 2 * g2 + u
                    nc.tensor.matmul(
                        s_ps[:, IC * u:IC * (u + 1)],
                        qks[k][DH * h:DH * (h + 1),
                               IC + 128 * jj:IC + 128 * (jj + 1)],
                        qt, start=True, stop=True)
                pt = pt_pool.tile([128, 2 * IC], bf16, name="pt")
                nc.scalar.activation(pt[:, :], s_ps[:, :], Exp, scale=SCALE)
                return pt

            def emit_attnv(state, g2):
                k, h = state["k"], state["h"]
                if g2 == 0:
                    state["acc4"] = psA.tile([128, IC], f32, tag="a",
                                             name="acc4")
                acc4 = state["acc4"]
                pt = state["pt"][g2]
                for u in range(2):
                    jj = 2 * g2 + u
                    for ib in range(4):
                        first = (g2 == 0 and u == 0 and ib == 0)
                        nc.tensor.matmul(
                            acc4[:, VW * ib:VW * (ib + 1)],
                            pt[:, IC * u + 128 * ib:IC * u + 128 * (ib + 1)],
                            vs[k][:, VW * (2 * jj + h):VW * (2 * jj + h + 1)],
                            start=first, stop=(g2 == 1 and u == 1),
                            skip_group_check=not first)

            def emit_finish(state, ot_tiles):
                k, c, h, pid = state["k"], state["c"], state["h"], state["pid"]
                acc4 = state["acc4"]
                nc.vector.tensor_add(parts[pid][:, 0:4 * VW],
                                     parts[pid][:, 0:4 * VW],
                                     acc4[:, 0:4 * VW])
                if k != NI - 1:
                    return
                # last sweep: normalize, transpose into O^T, then (h==1) the
                # partial output projection for this chunk
                if h == 0:
                    ot_tiles[c] = ot_pool.tile([128, IC], bf16, name="ot")
                ot_cur = ot_tiles[c]
                rcp4 = r_pool.tile([128, 4], f32, name="rcp4")
                nc.vector.reciprocal(
                    rcp4[:, :],
                    parts[pid].rearrange("p (b w) -> p b w", w=VW)[:, :,
                                                                  DH:DH + 1])
                for ib in range(4):
                    o_sb = o_pool.tile([128, DH], bf16, name="o_sb")
                    nc.vector.tensor_scalar_mul(
                        o_sb[:, :], parts[pid][:, VW * ib:VW * ib + DH],
                        rcp4[:, ib:ib + 1])
                    tr = psA.tile([DH, 128], bf16, tag="a", name="tr")
                    nc.tensor.transpose(tr[:, :], o_sb[:, :], id_sb[:, :])
                    nc.vector.tensor_copy(
                        ot_cur[DH * h:DH * (h + 1), 128 * ib:128 * (ib + 1)],
                        tr[:, :])
                if h == 1:
                    # one combined y tile + a single 3D-AP DMA per chunk
                    y_sb = y_pool.tile([128, 4 * QD], bf16, name="y_sb")
                    for ib in range(4):
                        for e in range(2):
                            y_ps = psA.tile([128, IC], f32, tag="a",
                                            name="y_ps")
                            nc.tensor.matmul(
                                y_ps[:, :], ot_cur[:, 128 * ib:128 * (ib + 1)],
                                wo_sb[:, IC * e:IC * (e + 1)],
                                start=True, stop=True)
                            dst = y_sb[:, QD * ib + IC * e:
                                       QD * ib + IC * (e + 1)]
                            # split PSUM->SBUF copies between DVE and Act:
                            # during the last sweep the DVE is the bottleneck
                            # while Act has slack; the final chunk drains
                            # after the last exp, so it all goes to Act
                            if e == 1 or c == NI - 1:
                                nc.scalar.copy(dst, y_ps[:, :])
                            else:
                                nc.vector.tensor_copy(dst, y_ps[:, :])
                    nc.sync.dma_start(
                        out=y_d[IC * c:IC * (c + 1), :].rearrange(
                            "(b p) w -> p b w", p=128),
                        in_=y_sb.rearrange("p (b w) -> p b w", w=QD))

            for _rep in range(repeat):
                ot_tiles = {}
                # prologue: only what the first score group needs — the first
                # half of K0 (j-blocks 0,1) and all of Q0; K0's second half,
                # V0 and later Q's trickle into the block stream
                proj_k_half(0, 0)
                proj_q(0)

                prev = None
                for p in range(NP * NI + 1):   # 128 pair blocks + 1 flush
                    cur = None
                    if p < NP * NI:
                        k, idx = divmod(p, NP)
                        c, h = divmod(idx, 2)
                        cur = {"k": k, "c": c, "h": h, "pid": idx, "pt": [None,
                                                                          None]}
                        cur["pt"][0] = emit_scores(k, c, h, 0)
                    if prev is not None:
                        emit_attnv(prev, 0)
                    if p < NP * NI:
                        # trickled projections (sweep 0 also carries K0's
                        # second half, V0, and the Q projections for chunks
                        # 1..7, each split across two blocks just ahead of
                        # first use)
                        if k == 0:
                            if idx == 0:
                                proj_k_half(0, 1)
                                for b in range(4):
                                    proj_v_piece(0, b)
                            cq = idx // 2 + 1
                            if cq < NI:
                                if idx % 2 == 0:
                                    proj_q_first(cq)
                                else:
                                    proj_q_second(cq)
                        if k < NI - 1:
                            kpos = (5, 7) if k == 0 else (4, 5)
                            vpos = (9, 11, 13, 15) if k == 0 else (8, 9, 10, 11)
                            if idx == kpos[0]:
                                proj_k_first(k + 1)
                            elif idx == kpos[1]:
                                proj_k_second(k + 1)
                            elif idx in vpos:
                                proj_v_piece(k + 1, vpos.index(idx))
                        cur["pt"][1] = emit_scores(k, c, h, 1)
                    if prev is not None:
                        emit_attnv(prev, 1)
                        emit_finish(prev, ot_tiles)
                    prev = cur
    nc.compile()
    return nc


def _get_nc():
    if "nc" not in _CACHE:
        _CACHE["nc"] = _build()
    return _CACHE["nc"]


def _in_maps(x, Wq, Wk, Wv, Wo):
    import ml_dtypes
    bf = ml_dtypes.bfloat16
    xt = np.ascontiguousarray(x.reshape(N, QD).T).astype(bf)
    ident = np.eye(128, dtype=np.float32).astype(bf)
    in_maps = []
    for k in range(N_CORES):
        cs = CPC * k
        in_maps.append({
            "xt": xt,
            "wq": np.ascontiguousarray(Wq[:, cs:cs + CPC]).astype(bf),
            "wk": np.ascontiguousarray(Wk[:, cs:cs + CPC]).astype(bf),
            "wv": np.ascontiguousarray(Wv[:, cs:cs + CPC]).astype(bf),
            "wo": np.ascontiguousarray(Wo[cs:cs + CPC, :]).astype(bf),
            "ident": ident,
        })
    return in_maps


def kernel(x, Wq, Wk, Wv, Wo, bo):
    from concourse.bass_utils import run_bass_kernel_spmd

    x = np.asarray(x, dtype=np.float32)
    Wq = np.asarray(Wq, dtype=np.float32)
    Wk = np.asarray(Wk, dtype=np.float32)
    Wv = np.asarray(Wv, dtype=np.float32)
    Wo = np.asarray(Wo, dtype=np.float32)
    bo = np.asarray(bo, dtype=np.float32)

    nc = _get_nc()
    res = run_bass_kernel_spmd(nc, _in_maps(x, Wq, Wk, Wv, Wo),
                               list(range(N_CORES)))
    y = np.zeros((N, QD), dtype=np.float32)
    for k in range(N_CORES):
        y += res.results[k]["y_out"].astype(np.float32)
    y = y + bo[None, :]
    return y.reshape(1, N, QD).astype(np.float32)
